# revision 9
# baseline (speedup 1.0000x reference)
"""Trainium2 Bass kernel for nn_AccuracyMetricLoss.

Computes mean over 200000 days of per-day scores:
    denom = max(t, 0.2*cap);  rel_sq = ((t-p)/denom)^2
    score_d = (1 - sqrt(mean_96(rel_sq))) * 100;  out = mean_d(score_d)

Sharding: day axis split evenly across 8 NeuronCores (25000 days/core).

Per-core pipeline, per [P, C_DAYS*96] tile (day-major, contiguous rows):
    DMA t (SP ring), p (ACT ring) -> SBUF
    DVE:  d = t - p              (in-place into p)
    ACT:  lt = ln(t); r2 = exp(-2*lt) = 1/t^2   (in-place into t; one table set)
    DVE:  custom fused op  s = cumsum(d^2 * min(r2, 1/thresh^2))  (into lt)
    GpSimd: copy strided per-day samples s[:, 95::96] into acc
    single final DMA of acc -> DRAM
Host: difference the prefix samples -> per-day sums, sqrt/score/mean in f64.
"""
import os
import sys

sys.path.insert(0, "/opt/trn_rl_repo")

import numpy as np

import concourse.bacc as bacc
import concourse.mybir as mybir
from concourse.bass_utils import run_bass_kernel_spmd
from concourse.tile import TileContext

from concourse.dve_ops import DveOp, OPS, CUSTOM_DVE_SPECS, _SUB_OPCODE_FOR_NAME
from concourse.dve_spec import Spec, Src0, Src1, C0, AluOp, sq, minn, scan, lower
from concourse.dve_uop import DveOpSpec

# ---------------- problem constants (hardcoded) ---------------- #
CAP = (300 + 400 + 900) / 300 / 1000 * 300400.0  # 1602.1333...
THRESH = np.float32(0.2) * np.float32(CAP)
C2 = float(1.0 / (np.float64(THRESH) ** 2))  # clamp for 1/t^2
T = 96
N_DAYS = 200000
N_CORES = 8
DAYS_PER_CORE = N_DAYS // N_CORES  # 25000
# DMA chunks are contiguous flat ranges reshaped [rows, 50 days]; all
# chunks stay resident in SBUF so loads are never gated on compute.
CHUNK_DAYS = 100  # days per partition row per chunk
CHUNK_FD = CHUNK_DAYS * T  # 9600
CHUNK_ROWS = [128, 122]  # 12800 + 12200 = 25000 days
N_CHUNKS = len(CHUNK_ROWS)
C_DAYS = 50  # days per row per compute slice (2 slices per chunk)
FD = C_DAYS * T  # 4800
SLICES_PER_CHUNK = CHUNK_FD // FD  # 2
P = 128
assert sum(r * CHUNK_DAYS for r in CHUNK_ROWS) == DAYS_PER_CORE


def _register_clamp_sq_scan():
    name = "CLAMP_SQ_SCAN_ANT"
    for op in OPS:
        if op.name == name:
            return op

    body = scan(AluOp.ADD, sq(Src0) * minn(Src1, C0))

    def _ref(in0, in1, s0, s1, imm2):
        x = np.asarray(in0, np.float32)
        r = np.asarray(in1, np.float32).reshape(x.shape[0], -1)
        c = s0 if isinstance(s0, float) else np.asarray(s0, np.float32).reshape(-1, 1)
        b = (x.reshape(x.shape[0], -1) ** 2) * np.minimum(r, c)
        out = np.cumsum(b.astype(np.float32), axis=-1, dtype=np.float32)
        return out.reshape(in0.shape)

    spec = Spec(body=body, reference=_ref)
    row = 1 + len(OPS)
    assert row < 0x20
    _SUB_OPCODE_FOR_NAME[name] = row
    shas = {}
    for ver in ("v3", "v4"):
        u = lower(spec, ver=ver)
        shas[ver] = DveOpSpec(name=name, opcode=row, uops=u, rd1_en=True).sha(ver)
    op = DveOp(name, spec, subdim=False, uops_sha=shas)
    OPS.append(op)
    CUSTOM_DVE_SPECS[name] = spec
    return op


def _pin_act_table_set():
    """Make Ln and Exp resolve only to natural_log_exp_and_others so the
    table-load pass emits one hoisted load instead of alternating reloads.
    Mutates the functools.cache'd dict in place (order/len preserved)."""
    from concourse.hw_specs import get_activation_tables

    tables = get_activation_tables("gen3")
    keep = "natural_log_exp_and_others"
    if keep not in tables:
        return
    for name, fns in tables.items():
        if name == keep:
            continue
        fns.discard(mybir.ActivationFunctionType.Ln)
        fns.discard(mybir.ActivationFunctionType.Exp)


_nc_cache = {}


def _build_nc():
    if "nc" in _nc_cache:
        return _nc_cache["nc"]
    clamp_sq_scan = _register_clamp_sq_scan()
    _pin_act_table_set()

    nc = bacc.Bacc("TRN2")
    n_elem = DAYS_PER_CORE * T
    t_in = nc.dram_tensor("t_in", [n_elem], mybir.dt.float32, kind="ExternalInput")
    p_in = nc.dram_tensor("p_in", [n_elem], mybir.dt.float32, kind="ExternalInput")
    out = nc.dram_tensor(
        "out",
        [P, N_CHUNKS * SLICES_PER_CHUNK * C_DAYS],
        mybir.dt.float32,
        kind="ExternalOutput",
    )
    with TileContext(nc) as tc:
        with (
            tc.tile_pool(name="tp", bufs=N_CHUNKS) as tp,
            tc.tile_pool(name="pp", bufs=N_CHUNKS) as pp,
            tc.tile_pool(name="lp", bufs=2) as lp,
            tc.tile_pool(name="accp", bufs=1) as accp,
        ):
            n_slices = N_CHUNKS * SLICES_PER_CHUNK
            acc = accp.tile([P, n_slices * C_DAYS], mybir.dt.float32)
            # all chunks stay resident: loads never gated on compute
            t_tiles, p_tiles = [], []
            base = 0
            for ci, rows in enumerate(CHUNK_ROWS):
                t = tp.tile([P, CHUNK_FD], mybir.dt.float32, tag="t")
                p = pp.tile([P, CHUNK_FD], mybir.dt.float32, tag="p")
                n = rows * CHUNK_FD
                t_v = t_in[base : base + n].rearrange("(p f) -> p f", p=rows)
                p_v = p_in[base : base + n].rearrange("(p f) -> p f", p=rows)
                eng = nc.sync if ci == 0 else nc.scalar
                eng.dma_start(out=t[:rows, :], in_=t_v)
                eng.dma_start(out=p[:rows, :], in_=p_v)
                t_tiles.append(t)
                p_tiles.append(p)
                base += n
            for c, rows in enumerate(CHUNK_ROWS):
                for j in range(SLICES_PER_CHUNK):
                    i = c * SLICES_PER_CHUNK + j  # global slice index
                    cols = slice(j * FD, (j + 1) * FD)
                    ts = t_tiles[c][:rows, cols]
                    ps = p_tiles[c][:rows, cols]
                    lt = lp.tile([P, FD], mybir.dt.float32, tag="lt")
                    lts = lt[:rows, :]
                    # lt = ln(t)
                    nc.scalar.activation(lts, ts, mybir.ActivationFunctionType.Ln)
                    # d = t - p   (in place into p)
                    nc.vector.tensor_tensor(ps, ts, ps, mybir.AluOpType.subtract)
                    # r2 = exp(-2*lt) = 1/t^2   (in place into t)
                    nc.scalar.activation(
                        ts, lts, mybir.ActivationFunctionType.Exp, scale=-2.0
                    )
                    # s = cumsum(d^2 * min(r2, C2))  (into lt)
                    nc.vector._custom_dve(
                        clamp_sq_scan, out=lts, in0=ps, in1=ts, s0=C2
                    )
                    # collect per-day prefix samples into acc
                    samples = lts.rearrange("p (c n) -> p c n", n=T)[:, :, 95]
                    nc.vector.tensor_copy(
                        acc[:rows, i * C_DAYS : (i + 1) * C_DAYS], samples
                    )
            nc.sync.dma_start(out=out[:], in_=acc[:])
    nc.finalize()
    _nc_cache["nc"] = nc
    return nc


_last_results = None


def kernel(pred: np.ndarray, true: np.ndarray) -> np.ndarray:
    global _last_results
    nc = _build_nc()

    n_elem = DAYS_PER_CORE * T
    pred = np.ascontiguousarray(pred, dtype=np.float32)
    true = np.ascontiguousarray(true, dtype=np.float32)
    in_maps = [
        {
            "t_in": true[k * n_elem : (k + 1) * n_elem],
            "p_in": pred[k * n_elem : (k + 1) * n_elem],
        }
        for k in range(N_CORES)
    ]

    trace = bool(os.environ.get("BASS_TRACE"))
    res = run_bass_kernel_spmd(nc, in_maps, list(range(N_CORES)), trace=trace)
    _last_results = res

    # host-side tail: prefix samples -> day sums -> scores -> mean
    n_slices = N_CHUNKS * SLICES_PER_CHUNK
    total = 0.0
    for k in range(N_CORES):
        A = res.results[k]["out"].astype(np.float64)  # [128, n_slices*C_DAYS]
        A = A.reshape(P, n_slices, C_DAYS)
        u = A.copy()
        u[:, :, 1:] -= A[:, :, :-1]  # per-day sums of rel_sq
        scores = (1.0 - np.sqrt(u / T)) * 100.0
        for c, rows in enumerate(CHUNK_ROWS):
            for j in range(SLICES_PER_CHUNK):
                total += scores[:rows, c * SLICES_PER_CHUNK + j, :].sum()
    return np.float32(total / N_DAYS)


# revision 12
# speedup vs baseline: 2.5905x; 2.5905x over previous
"""Trainium2 Bass kernel for nn_AccuracyMetricLoss.

Computes mean over 200000 days of per-day scores:
    denom = max(t, 0.2*cap);  rel_sq = ((t-p)/denom)^2
    score_d = (1 - sqrt(mean_96(rel_sq))) * 100;  out = mean_d(score_d)

Sharding: day axis split evenly across 8 NeuronCores (25000 days/core).

Per-core pipeline: inputs stream in as a few large DRAM-sequential chunks
(all on the SP HWDGE ring; all chunks stay resident in SBUF so loads are
never gated on compute). Per day-aligned compute slice:
    ACT:  lt = ln(t); r2 = exp(-2*lt) = 1/t^2   (in-place into t; one table set)
    DVE:  d = t - p                             (in-place into p)
    DVE:  custom fused op  s = cumsum(d^2 * min(r2, 1/thresh^2))
    DVE:  copy strided per-day prefix samples s[:, 95::96] into acc
    one final DMA of acc -> DRAM
Host: difference the prefix samples -> per-day sums, sqrt/score/mean in f64.
"""
import os
import sys

sys.path.insert(0, "/opt/trn_rl_repo")

import numpy as np

import concourse.bacc as bacc
import concourse.mybir as mybir
from concourse.bass_utils import run_bass_kernel_spmd
from concourse.tile import TileContext

from concourse.dve_ops import DveOp, OPS, CUSTOM_DVE_SPECS, _SUB_OPCODE_FOR_NAME
from concourse.dve_spec import Spec, Src0, Src1, C0, AluOp, sq, minn, scan, lower
from concourse.dve_uop import DveOpSpec

# ---------------- problem constants (hardcoded) ---------------- #
CAP = (300 + 400 + 900) / 300 / 1000 * 300400.0  # 1602.1333...
THRESH = np.float32(0.2) * np.float32(CAP)
C2 = float(1.0 / (np.float64(THRESH) ** 2))  # clamp for 1/t^2
T = 96
N_DAYS = 200000
N_CORES = 8
DAYS_PER_CORE = N_DAYS // N_CORES  # 25000
P = 128
# DMA chunks: contiguous flat ranges reshaped [rows, days_per_row*96].
# (rows, days_per_row, [compute slice day-widths])
CHUNKS = [
    (128, 56, [28, 28]),
    (128, 100, [50, 50]),
    (128, 39, [39]),
    (8, 5, [5]),
]
assert sum(r * c for r, c, _ in CHUNKS) == DAYS_PER_CORE
for _r, _c, _s in CHUNKS:
    assert sum(_s) == _c
ACC_COLS = sum(sum(s) for _, _, s in CHUNKS)  # 200
MAX_SLICE_FD = max(s for _, _, sl in CHUNKS for s in sl) * T  # 4800


def _register_clamp_sq_scan():
    name = "CLAMP_SQ_SCAN_ANT"
    for op in OPS:
        if op.name == name:
            return op

    body = scan(AluOp.ADD, sq(Src0) * minn(Src1, C0))

    def _ref(in0, in1, s0, s1, imm2):
        x = np.asarray(in0, np.float32)
        r = np.asarray(in1, np.float32).reshape(x.shape[0], -1)
        c = s0 if isinstance(s0, float) else np.asarray(s0, np.float32).reshape(-1, 1)
        b = (x.reshape(x.shape[0], -1) ** 2) * np.minimum(r, c)
        out = np.cumsum(b.astype(np.float32), axis=-1, dtype=np.float32)
        return out.reshape(in0.shape)

    spec = Spec(body=body, reference=_ref)
    row = 1 + len(OPS)
    assert row < 0x20
    _SUB_OPCODE_FOR_NAME[name] = row
    shas = {}
    for ver in ("v3", "v4"):
        u = lower(spec, ver=ver)
        shas[ver] = DveOpSpec(name=name, opcode=row, uops=u, rd1_en=True).sha(ver)
    op = DveOp(name, spec, subdim=False, uops_sha=shas)
    OPS.append(op)
    CUSTOM_DVE_SPECS[name] = spec
    return op


def _pin_act_table_set():
    """Make Ln and Exp resolve only to natural_log_exp_and_others so the
    table-load pass emits one hoisted load instead of alternating reloads.
    Mutates the functools.cache'd dict in place (order/len preserved)."""
    from concourse.hw_specs import get_activation_tables

    tables = get_activation_tables("gen3")
    keep = "natural_log_exp_and_others"
    if keep not in tables:
        return
    for name, fns in tables.items():
        if name == keep:
            continue
        fns.discard(mybir.ActivationFunctionType.Ln)
        fns.discard(mybir.ActivationFunctionType.Exp)


_nc_cache = {}


def _build_nc():
    if "nc" in _nc_cache:
        return _nc_cache["nc"]
    clamp_sq_scan = _register_clamp_sq_scan()
    _pin_act_table_set()

    nc = bacc.Bacc("TRN2")
    n_elem = DAYS_PER_CORE * T
    t_in = nc.dram_tensor("t_in", [n_elem], mybir.dt.float32, kind="ExternalInput")
    p_in = nc.dram_tensor("p_in", [n_elem], mybir.dt.float32, kind="ExternalInput")
    out = nc.dram_tensor("out", [P, ACC_COLS], mybir.dt.float32, kind="ExternalOutput")

    with TileContext(nc) as tc:
        with (
            tc.tile_pool(name="tp", bufs=1) as tp,
            tc.tile_pool(name="pp", bufs=1) as pp,
            tc.tile_pool(name="lp", bufs=2) as lp,
            tc.tile_pool(name="accp", bufs=1) as accp,
        ):
            acc = accp.tile([P, ACC_COLS], mybir.dt.float32)
            # all chunks stay resident: loads never gated on compute;
            # single ring (SP), ordered t0,p0,t1,p1,... so early chunks
            # complete first and compute streams behind the loads
            t_tiles, p_tiles = [], []
            base = 0
            for ci, (rows, cdays, _) in enumerate(CHUNKS):
                fd = cdays * T
                t = tp.tile([P, fd], mybir.dt.float32, tag=f"t{ci}")
                p = pp.tile([P, fd], mybir.dt.float32, tag=f"p{ci}")
                n = rows * fd
                t_v = t_in[base : base + n].rearrange("(p f) -> p f", p=rows)
                p_v = p_in[base : base + n].rearrange("(p f) -> p f", p=rows)
                nc.sync.dma_start(out=t[:rows, :], in_=t_v)
                nc.sync.dma_start(out=p[:rows, :], in_=p_v)
                t_tiles.append(t)
                p_tiles.append(p)
                base += n
            acc_col = 0
            for ci, (rows, cdays, slices) in enumerate(CHUNKS):
                off = 0
                for sdays in slices:
                    fd = sdays * T
                    cols = slice(off * T, off * T + fd)
                    ts = t_tiles[ci][:rows, cols]
                    ps = p_tiles[ci][:rows, cols]
                    lt = lp.tile([P, MAX_SLICE_FD], mybir.dt.float32, tag="lt")
                    lts = lt[:rows, :fd]
                    # lt = ln(t)
                    nc.scalar.activation(lts, ts, mybir.ActivationFunctionType.Ln)
                    # d = t - p   (in place into p)
                    nc.vector.tensor_tensor(ps, ts, ps, mybir.AluOpType.subtract)
                    # r2 = exp(-2*lt) = 1/t^2   (in place into t)
                    nc.scalar.activation(
                        ts, lts, mybir.ActivationFunctionType.Exp, scale=-2.0
                    )
                    # s = cumsum(d^2 * min(r2, C2))  (into lt)
                    nc.vector._custom_dve(
                        clamp_sq_scan, out=lts, in0=ps, in1=ts, s0=C2
                    )
                    # collect per-day prefix samples into acc
                    samples = lts.rearrange("p (c n) -> p c n", n=T)[:, :, 95]
                    nc.vector.tensor_copy(
                        acc[:rows, acc_col : acc_col + sdays], samples
                    )
                    off += sdays
                    acc_col += sdays
            nc.sync.dma_start(out=out[:], in_=acc[:])
    nc.finalize()
    _nc_cache["nc"] = nc
    return nc


_last_results = None


def kernel(pred: np.ndarray, true: np.ndarray) -> np.ndarray:
    global _last_results
    nc = _build_nc()

    n_elem = DAYS_PER_CORE * T
    pred = np.ascontiguousarray(pred, dtype=np.float32)
    true = np.ascontiguousarray(true, dtype=np.float32)
    in_maps = [
        {
            "t_in": true[k * n_elem : (k + 1) * n_elem],
            "p_in": pred[k * n_elem : (k + 1) * n_elem],
        }
        for k in range(N_CORES)
    ]

    trace = bool(os.environ.get("BASS_TRACE"))
    res = run_bass_kernel_spmd(nc, in_maps, list(range(N_CORES)), trace=trace)
    _last_results = res

    # host-side tail: prefix samples -> day sums -> scores -> mean
    total = 0.0
    for k in range(N_CORES):
        A = res.results[k]["out"].astype(np.float64)  # [128, ACC_COLS]
        acc_col = 0
        for rows, cdays, slices in CHUNKS:
            for sdays in slices:
                S = A[:rows, acc_col : acc_col + sdays]
                u = S.copy()
                u[:, 1:] -= S[:, :-1]  # per-day sums of rel_sq
                scores = (1.0 - np.sqrt(u / T)) * 100.0
                total += scores.sum()
                acc_col += sdays
    return np.float32(total / N_DAYS)


# revision 13
# speedup vs baseline: 3.3815x; 1.3053x over previous
"""Trainium2 Bass kernel for nn_AccuracyMetricLoss.

Computes mean over 200000 days of per-day scores:
    denom = max(t, 0.2*cap);  rel_sq = ((t-p)/denom)^2
    score_d = (1 - sqrt(mean_96(rel_sq))) * 100;  out = mean_d(score_d)

Sharding: day axis split evenly across 8 NeuronCores (25000 days/core).

Per-core pipeline: inputs stream in as a few large DRAM-sequential chunks
(all on the SP HWDGE ring; all chunks stay resident in SBUF so loads are
never gated on compute). Per day-aligned compute slice:
    ACT:  lt = ln(t); r2 = exp(-2*lt) = 1/t^2   (in-place into t; one table set)
    DVE:  d = t - p                             (in-place into p)
    DVE:  custom fused op  s = cumsum(d^2 * min(r2, 1/thresh^2))
    DVE:  copy strided per-day prefix samples s[:, 95::96] into acc
    one final DMA of acc -> DRAM
Host: difference the prefix samples -> per-day sums, sqrt/score/mean in f64.
"""
import os
import sys

sys.path.insert(0, "/opt/trn_rl_repo")

import numpy as np

import concourse.bacc as bacc
import concourse.mybir as mybir
from concourse.bass_utils import run_bass_kernel_spmd
from concourse.tile import TileContext

from concourse.dve_ops import DveOp, OPS, CUSTOM_DVE_SPECS, _SUB_OPCODE_FOR_NAME
from concourse.dve_spec import Spec, Src0, Src1, C0, AluOp, sq, minn, scan, lower
from concourse.dve_uop import DveOpSpec

# ---------------- problem constants (hardcoded) ---------------- #
CAP = (300 + 400 + 900) / 300 / 1000 * 300400.0  # 1602.1333...
THRESH = np.float32(0.2) * np.float32(CAP)
C2 = float(1.0 / (np.float64(THRESH) ** 2))  # clamp for 1/t^2
T = 96
N_DAYS = 200000
N_CORES = 8
DAYS_PER_CORE = N_DAYS // N_CORES  # 25000
P = 128
# DMA chunks: contiguous flat ranges reshaped [rows, days_per_row*96].
# (rows, days_per_row, [compute slice day-widths])
CHUNKS = [
    (128, 40, [40]),
    (128, 55, [28, 27]),
    (128, 60, [30, 30]),
    (128, 40, [40]),
    (8, 5, [5]),
]
assert sum(r * c for r, c, _ in CHUNKS) == DAYS_PER_CORE
for _r, _c, _s in CHUNKS:
    assert sum(_s) == _c
ACC_COLS = sum(sum(s) for _, _, s in CHUNKS)  # 200
MAX_SLICE_FD = max(s for _, _, sl in CHUNKS for s in sl) * T  # 4800


def _register_clamp_sq_scan():
    name = "CLAMP_SQ_SCAN_ANT"
    for op in OPS:
        if op.name == name:
            return op

    body = scan(AluOp.ADD, sq(Src0) * minn(Src1, C0))

    def _ref(in0, in1, s0, s1, imm2):
        x = np.asarray(in0, np.float32)
        r = np.asarray(in1, np.float32).reshape(x.shape[0], -1)
        c = s0 if isinstance(s0, float) else np.asarray(s0, np.float32).reshape(-1, 1)
        b = (x.reshape(x.shape[0], -1) ** 2) * np.minimum(r, c)
        out = np.cumsum(b.astype(np.float32), axis=-1, dtype=np.float32)
        return out.reshape(in0.shape)

    spec = Spec(body=body, reference=_ref)
    row = 1 + len(OPS)
    assert row < 0x20
    _SUB_OPCODE_FOR_NAME[name] = row
    shas = {}
    for ver in ("v3", "v4"):
        u = lower(spec, ver=ver)
        shas[ver] = DveOpSpec(name=name, opcode=row, uops=u, rd1_en=True).sha(ver)
    op = DveOp(name, spec, subdim=False, uops_sha=shas)
    OPS.append(op)
    CUSTOM_DVE_SPECS[name] = spec
    return op


def _pin_act_table_set():
    """Make Ln and Exp resolve only to natural_log_exp_and_others so the
    table-load pass emits one hoisted load instead of alternating reloads.
    Mutates the functools.cache'd dict in place (order/len preserved)."""
    from concourse.hw_specs import get_activation_tables

    tables = get_activation_tables("gen3")
    keep = "natural_log_exp_and_others"
    if keep not in tables:
        return
    for name, fns in tables.items():
        if name == keep:
            continue
        fns.discard(mybir.ActivationFunctionType.Ln)
        fns.discard(mybir.ActivationFunctionType.Exp)


_nc_cache = {}


def _build_nc():
    if "nc" in _nc_cache:
        return _nc_cache["nc"]
    clamp_sq_scan = _register_clamp_sq_scan()
    _pin_act_table_set()

    nc = bacc.Bacc("TRN2")
    n_elem = DAYS_PER_CORE * T
    t_in = nc.dram_tensor("t_in", [n_elem], mybir.dt.float32, kind="ExternalInput")
    p_in = nc.dram_tensor("p_in", [n_elem], mybir.dt.float32, kind="ExternalInput")
    out = nc.dram_tensor("out", [P, ACC_COLS], mybir.dt.float32, kind="ExternalOutput")

    with TileContext(nc) as tc:
        with (
            tc.tile_pool(name="tp", bufs=1) as tp,
            tc.tile_pool(name="pp", bufs=1) as pp,
            tc.tile_pool(name="lp", bufs=2) as lp,
            tc.tile_pool(name="accp", bufs=1) as accp,
        ):
            acc = accp.tile([P, ACC_COLS], mybir.dt.float32)
            # all chunks stay resident: loads never gated on compute;
            # single ring (SP), ordered t0,p0,t1,p1,... so early chunks
            # complete first and compute streams behind the loads
            t_tiles, p_tiles = [], []
            base = 0
            for ci, (rows, cdays, _) in enumerate(CHUNKS):
                fd = cdays * T
                t = tp.tile([P, fd], mybir.dt.float32, tag=f"t{ci}")
                p = pp.tile([P, fd], mybir.dt.float32, tag=f"p{ci}")
                n = rows * fd
                t_v = t_in[base : base + n].rearrange("(p f) -> p f", p=rows)
                p_v = p_in[base : base + n].rearrange("(p f) -> p f", p=rows)
                nc.sync.dma_start(out=t[:rows, :], in_=t_v)
                nc.sync.dma_start(out=p[:rows, :], in_=p_v)
                t_tiles.append(t)
                p_tiles.append(p)
                base += n
            acc_col = 0
            for ci, (rows, cdays, slices) in enumerate(CHUNKS):
                off = 0
                for sdays in slices:
                    fd = sdays * T
                    cols = slice(off * T, off * T + fd)
                    ts = t_tiles[ci][:rows, cols]
                    ps = p_tiles[ci][:rows, cols]
                    lt = lp.tile([P, MAX_SLICE_FD], mybir.dt.float32, tag="lt")
                    lts = lt[:rows, :fd]
                    # lt = ln(t)
                    nc.scalar.activation(lts, ts, mybir.ActivationFunctionType.Ln)
                    # d = t - p   (in place into p)
                    nc.vector.tensor_tensor(ps, ts, ps, mybir.AluOpType.subtract)
                    # r2 = exp(-2*lt) = 1/t^2   (in place into t)
                    nc.scalar.activation(
                        ts, lts, mybir.ActivationFunctionType.Exp, scale=-2.0
                    )
                    # s = cumsum(d^2 * min(r2, C2))  (into lt)
                    nc.vector._custom_dve(
                        clamp_sq_scan, out=lts, in0=ps, in1=ts, s0=C2
                    )
                    # collect per-day prefix samples into acc
                    samples = lts.rearrange("p (c n) -> p c n", n=T)[:, :, 95]
                    nc.vector.tensor_copy(
                        acc[:rows, acc_col : acc_col + sdays], samples
                    )
                    off += sdays
                    acc_col += sdays
            nc.sync.dma_start(out=out[:], in_=acc[:])
    nc.finalize()
    _nc_cache["nc"] = nc
    return nc


_last_results = None


def kernel(pred: np.ndarray, true: np.ndarray) -> np.ndarray:
    global _last_results
    nc = _build_nc()

    n_elem = DAYS_PER_CORE * T
    pred = np.ascontiguousarray(pred, dtype=np.float32)
    true = np.ascontiguousarray(true, dtype=np.float32)
    in_maps = [
        {
            "t_in": true[k * n_elem : (k + 1) * n_elem],
            "p_in": pred[k * n_elem : (k + 1) * n_elem],
        }
        for k in range(N_CORES)
    ]

    trace = bool(os.environ.get("BASS_TRACE"))
    res = run_bass_kernel_spmd(nc, in_maps, list(range(N_CORES)), trace=trace)
    _last_results = res

    # host-side tail: prefix samples -> day sums -> scores -> mean
    total = 0.0
    for k in range(N_CORES):
        A = res.results[k]["out"].astype(np.float64)  # [128, ACC_COLS]
        acc_col = 0
        for rows, cdays, slices in CHUNKS:
            for sdays in slices:
                S = A[:rows, acc_col : acc_col + sdays]
                u = S.copy()
                u[:, 1:] -= S[:, :-1]  # per-day sums of rel_sq
                scores = (1.0 - np.sqrt(u / T)) * 100.0
                total += scores.sum()
                acc_col += sdays
    return np.float32(total / N_DAYS)


# revision 14
# speedup vs baseline: 3.6345x; 1.0748x over previous
"""Trainium2 Bass kernel for nn_AccuracyMetricLoss.

Computes mean over 200000 days of per-day scores:
    denom = max(t, 0.2*cap);  rel_sq = ((t-p)/denom)^2
    score_d = (1 - sqrt(mean_96(rel_sq))) * 100;  out = mean_d(score_d)

Sharding: day axis split evenly across 8 NeuronCores (25000 days/core).

Per-core pipeline: inputs stream in as a few large DRAM-sequential chunks
(all on the SP HWDGE ring; all chunks stay resident in SBUF so loads are
never gated on compute). Per day-aligned compute slice:
    ACT:  lt = ln(t); r2 = exp(-2*lt) = 1/t^2   (in-place into t; one table set)
    DVE:  d = t - p                             (in-place into p)
    DVE:  custom fused op  s = cumsum(d^2 * min(r2, 1/thresh^2))
    DVE:  copy strided per-day prefix samples s[:, 95::96] into acc
    one final DMA of acc -> DRAM
Host: difference the prefix samples -> per-day sums, sqrt/score/mean in f64.
"""
import os
import sys

sys.path.insert(0, "/opt/trn_rl_repo")

import numpy as np

import concourse.bacc as bacc
import concourse.mybir as mybir
from concourse.bass_utils import run_bass_kernel_spmd
from concourse.tile import TileContext

from concourse.dve_ops import DveOp, OPS, CUSTOM_DVE_SPECS, _SUB_OPCODE_FOR_NAME
from concourse.dve_spec import Spec, Src0, Src1, C0, AluOp, sq, minn, scan, lower
from concourse.dve_uop import DveOpSpec

# ---------------- problem constants (hardcoded) ---------------- #
CAP = (300 + 400 + 900) / 300 / 1000 * 300400.0  # 1602.1333...
THRESH = np.float32(0.2) * np.float32(CAP)
C2 = float(1.0 / (np.float64(THRESH) ** 2))  # clamp for 1/t^2
CQ = float(np.float64(THRESH) ** -0.5)  # clamp for q = t^-1/2  (q^4 = 1/t^2)
T = 96
N_DAYS = 200000
N_CORES = 8
DAYS_PER_CORE = N_DAYS // N_CORES  # 25000
P = 128
# DMA chunks: contiguous flat ranges reshaped [rows, days_per_row*96].
# (rows, days_per_row, [compute slice day-widths])
CHUNKS = [
    (128, 25, [25]),
    (128, 50, [50]),
    (128, 55, [28, 27]),
    (128, 45, [45]),
    (128, 20, [20]),
    (8, 5, [5]),
]
assert sum(r * c for r, c, _ in CHUNKS) == DAYS_PER_CORE
for _r, _c, _s in CHUNKS:
    assert sum(_s) == _c
ACC_COLS = sum(sum(s) for _, _, s in CHUNKS)  # 200
MAX_SLICE_FD = max(s for _, _, sl in CHUNKS for s in sl) * T  # 4800


def _register_clamp_sq_scan():
    # out = cumsum(in0^2 * min(in1, s0)^4): in0 = t-p, in1 = t^-1/2,
    # s0 = thresh^-1/2, so min(in1,s0)^4 = 1/max(t,thresh)^2
    name = "CLAMP4_SQ_SCAN_ANT"
    for op in OPS:
        if op.name == name:
            return op

    qc = minn(Src1, C0)
    body = scan(AluOp.ADD, sq(Src0) * sq(sq(qc)))

    def _ref(in0, in1, s0, s1, imm2):
        x = np.asarray(in0, np.float32)
        r = np.asarray(in1, np.float32).reshape(x.shape[0], -1)
        c = s0 if isinstance(s0, float) else np.asarray(s0, np.float32).reshape(-1, 1)
        b = (x.reshape(x.shape[0], -1) ** 2) * np.minimum(r, c) ** 4
        out = np.cumsum(b.astype(np.float32), axis=-1, dtype=np.float32)
        return out.reshape(in0.shape)

    spec = Spec(body=body, reference=_ref)
    row = 1 + len(OPS)
    assert row < 0x20
    _SUB_OPCODE_FOR_NAME[name] = row
    shas = {}
    for ver in ("v3", "v4"):
        u = lower(spec, ver=ver)
        shas[ver] = DveOpSpec(name=name, opcode=row, uops=u, rd1_en=True).sha(ver)
    op = DveOp(name, spec, subdim=False, uops_sha=shas)
    OPS.append(op)
    CUSTOM_DVE_SPECS[name] = spec
    return op


def _pin_act_table_set():
    """Make Ln and Exp resolve only to natural_log_exp_and_others so the
    table-load pass emits one hoisted load instead of alternating reloads.
    Mutates the functools.cache'd dict in place (order/len preserved)."""
    from concourse.hw_specs import get_activation_tables

    tables = get_activation_tables("gen3")
    keep = "natural_log_exp_and_others"
    if keep not in tables:
        return
    for name, fns in tables.items():
        if name == keep:
            continue
        fns.discard(mybir.ActivationFunctionType.Ln)
        fns.discard(mybir.ActivationFunctionType.Exp)


_nc_cache = {}


def _build_nc():
    if "nc" in _nc_cache:
        return _nc_cache["nc"]
    clamp_sq_scan = _register_clamp_sq_scan()

    nc = bacc.Bacc("TRN2")
    n_elem = DAYS_PER_CORE * T
    t_in = nc.dram_tensor("t_in", [n_elem], mybir.dt.float32, kind="ExternalInput")
    p_in = nc.dram_tensor("p_in", [n_elem], mybir.dt.float32, kind="ExternalInput")
    out = nc.dram_tensor("out", [P, ACC_COLS], mybir.dt.float32, kind="ExternalOutput")

    with TileContext(nc) as tc:
        with (
            tc.tile_pool(name="tp", bufs=1) as tp,
            tc.tile_pool(name="pp", bufs=1) as pp,
            tc.tile_pool(name="lp", bufs=2) as lp,
            tc.tile_pool(name="accp", bufs=1) as accp,
        ):
            acc = accp.tile([P, ACC_COLS], mybir.dt.float32)
            # all chunks stay resident: loads never gated on compute;
            # single ring (SP), ordered t0,p0,t1,p1,... so early chunks
            # complete first and compute streams behind the loads
            t_tiles, p_tiles = [], []
            base = 0
            for ci, (rows, cdays, _) in enumerate(CHUNKS):
                fd = cdays * T
                t = tp.tile([P, fd], mybir.dt.float32, tag=f"t{ci}")
                p = pp.tile([P, fd], mybir.dt.float32, tag=f"p{ci}")
                n = rows * fd
                t_v = t_in[base : base + n].rearrange("(p f) -> p f", p=rows)
                p_v = p_in[base : base + n].rearrange("(p f) -> p f", p=rows)
                nc.sync.dma_start(out=t[:rows, :], in_=t_v)
                nc.sync.dma_start(out=p[:rows, :], in_=p_v)
                t_tiles.append(t)
                p_tiles.append(p)
                base += n
            acc_col = 0
            for ci, (rows, cdays, slices) in enumerate(CHUNKS):
                off = 0
                for sdays in slices:
                    fd = sdays * T
                    cols = slice(off * T, off * T + fd)
                    ts = t_tiles[ci][:rows, cols]
                    ps = p_tiles[ci][:rows, cols]
                    lt = lp.tile([P, MAX_SLICE_FD], mybir.dt.float32, tag="lt")
                    lts = lt[:rows, :fd]
                    # q = t^-1/2
                    nc.scalar.activation(
                        lts, ts, mybir.ActivationFunctionType.Abs_reciprocal_sqrt
                    )
                    # d = t - p   (in place into p)
                    nc.vector.tensor_tensor(ps, ts, ps, mybir.AluOpType.subtract)
                    # s = cumsum(d^2 * min(q, CQ)^4)  (into t: dead after sub)
                    nc.vector._custom_dve(
                        clamp_sq_scan, out=ts, in0=ps, in1=lts, s0=CQ
                    )
                    # collect per-day prefix samples into acc
                    samples = ts.rearrange("p (c n) -> p c n", n=T)[:, :, 95]
                    nc.vector.tensor_copy(
                        acc[:rows, acc_col : acc_col + sdays], samples
                    )
                    off += sdays
                    acc_col += sdays
            nc.sync.dma_start(out=out[:], in_=acc[:])
    nc.finalize()
    _nc_cache["nc"] = nc
    return nc


_last_results = None


def kernel(pred: np.ndarray, true: np.ndarray) -> np.ndarray:
    global _last_results
    nc = _build_nc()

    n_elem = DAYS_PER_CORE * T
    pred = np.ascontiguousarray(pred, dtype=np.float32)
    true = np.ascontiguousarray(true, dtype=np.float32)
    in_maps = [
        {
            "t_in": true[k * n_elem : (k + 1) * n_elem],
            "p_in": pred[k * n_elem : (k + 1) * n_elem],
        }
        for k in range(N_CORES)
    ]

    trace = bool(os.environ.get("BASS_TRACE"))
    res = run_bass_kernel_spmd(nc, in_maps, list(range(N_CORES)), trace=trace)
    _last_results = res

    # host-side tail: prefix samples -> day sums -> scores -> mean
    total = 0.0
    for k in range(N_CORES):
        A = res.results[k]["out"].astype(np.float64)  # [128, ACC_COLS]
        acc_col = 0
        for rows, cdays, slices in CHUNKS:
            for sdays in slices:
                S = A[:rows, acc_col : acc_col + sdays]
                u = S.copy()
                u[:, 1:] -= S[:, :-1]  # per-day sums of rel_sq
                scores = (1.0 - np.sqrt(u / T)) * 100.0
                total += scores.sum()
                acc_col += sdays
    return np.float32(total / N_DAYS)
